# revision 37
# baseline (speedup 1.0000x reference)
"""GPTQ/ExLlama 4-bit grouped-quantized linear on 8 Trainium2 NeuronCores.

out = x @ dequant(qweight, qzeros, scales) + bias
  x: [4, 2048, 4096] fp16, qweight: [512, 4096] int32 (8 nibbles/int32 along K),
  qzeros: [32, 512] int32 (8 nibbles/int32 along N), scales: [32, 4096] fp16,
  g_idx = arange(K)//128, bias: [4096] fp16.

Sharding: Megatron column-parallel. Each of the 8 cores gets the full x
(replicated) and a 512-wide column slice of qweight/zeros/scales/bias, computes
out[:, n_slice] = x @ W[:, n_slice] + bias[n_slice]; the host concatenates.

Host prep (layout only): nibbles are pre-extracted to uint8 (qb[p, g, n] =
nibble k=g*128+p of column n) so the device does no shift/mask work, and the
zeros are folded as t = (z+1)*s so dequant is the 3-op chain
  w = cast_f16(q) * s - t
with s/t host-replicated across partitions (plain fast linear loads).

Per-core kernel schedule. Steady state runs at the PE roofline (216 ns per
128x128x512 fp16 matmul, LDWEIGHTS hidden by the reorder window); startup is
dominated by the tile scheduler's DMA-transpose deadlock guard, which
bidirectionally serializes ANY concurrently-scheduled DMA against an
in-flight transpose (and a waiting DMA head-of-line blocks its whole engine
queue). Hence:
  - sync (SP) HWDGE ring, strict FIFO: xt0 transpose, then ALL dequant
    inputs (qb/zs interleaved per super-chunk, then bias), then xt1..31.
    Same-ring ordering costs only bandwidth, never a guard cascade.
  - stores ride SWDGE (gpsimd): they serialize one-way against transposes
    (the store waits; the transpose train is unaffected). Last store rides
    sync so its completion isn't queued behind SWDGE at the kernel tail.
  - PE: the first two m-tiles' matmuls are emitted super-chunk-interleaved
    (accumulate k-chunks 4sc..4sc+3 across the in-flight PSUM banks as W
    chunks dequantize) so the PE absorbs the dequant tail with real work;
    remaining m-tiles in plain k-ascending order.
  - DVE: u8->f16 casts first (qb-gated), then mul/sub chains (zs-gated),
    then per-tile bias adds during the PSUM->SBUF drain.
fp8 DoubleRow was tried for the last 6 k-chunks (rel err 1.65e-2, passes) but
measured at parity with fp16 (DoubleRow LDWEIGHTS can't pull ahead), so it
stays disabled (R_FP8 = 0).
"""

import os
import sys

for _p in ("/opt/trn_rl_repo", "/root/.axon_site/_ro/trn_rl_repo"):
    if os.path.isdir(_p) and _p not in sys.path:
        sys.path.insert(0, _p)

import numpy as np

import concourse.bass as bass
import concourse.mybir as mybir
import concourse.tile as tile
from concourse.bass_utils import run_bass_kernel_spmd

P = 128                    # partitions
B, S, K, N = 4, 2048, 4096, 4096
M = B * S                  # 8192 rows
GS = 128                   # quant group size (== one k-chunk)
G = K // GS                # 32 groups == k-chunks
NCORES = 8
NC = N // NCORES           # 512 output cols per core
SC = 4                     # groups per dequant super-chunk
NSC = G // SC              # 8 super-chunks
MT = 256                   # x rows per transposed DMA load
NMT = M // MT              # 32 loads
MSUB = MT // P             # 2 psum tiles per load
NWARM = 24                 # PE warm-up matmuls

# fp8 hybrid: the last 2*R_FP8 k-chunks run as e4m3 DoubleRow matmul pairs.
# Measured end-to-end rel-err vs the reference (seed-0): R=0: 5.7e-4,
# R=3: 1.65e-2 (passes the 2e-2 gate) — but on HW a DoubleRow pair costs
# ~375-430 ns vs 432 ns for the two fp16 matmuls it replaces (the
# interleaved LDWEIGHTS can't pull ahead into the background weight buffer,
# so its 213 ns serializes). No net win, so keep fp16 everywhere.
R_FP8 = 0
G8 = 2 * R_FP8             # fp8 chunks
G16 = G - G8               # fp16 chunks

_built = None


def _split_multiwaits(nc):
    """This container's walrus rejects any instruction carrying more than one
    semaphore wait ("Too many sync wait commands"). Hoist all but one wait of
    each multi-wait instruction into standalone EventSemaphore (wait-only)
    instructions on the same engine, inserted immediately before it — the
    engine queue is FIFO, so semantics are identical."""
    n = 0
    for fn in nc.m.functions:
        for blk in fn.blocks:
            out = []
            for inst in blk.instructions:
                si = getattr(inst, "sync_info", None)
                waits = list(si.on_wait) if si is not None and si.on_wait else []
                if len(waits) > 1:
                    for k, w in enumerate(waits[:-1]):
                        es = mybir.InstEventSemaphore(
                            name=f"{inst.name}.hoistw{k}", ins=[], outs=[],
                            sync_info=mybir.SyncInfo(on_wait=[w], on_update=[]),
                        )
                        es.engine = inst.engine
                        out.append(es)
                        n += 1
                    si.on_wait = [waits[-1]]
                out.append(inst)
            blk.instructions = out
    return n


def _build_bass():
    """Build the (identical-per-core) Bass program once."""
    global _built
    if _built is not None:
        return _built

    nc = bass.Bass()
    x_h = nc.dram_tensor("x", [M, K], mybir.dt.float16, kind="ExternalInput")
    qb_h = nc.dram_tensor("qb", [P, G * NC], mybir.dt.uint8, kind="ExternalInput")
    # zs/bias come host-replicated across partitions. All dequant inputs ride
    # the SAME ring as the x transposes, in FIFO order right after xt0: any
    # DMA scheduled on another ring while a transpose is in flight gets
    # serialized against that whole transpose by the tile scheduler's
    # deadlock guard (and a waiting DMA head-of-line blocks its engine's
    # entire queue), which is far worse than plain bandwidth sharing.
    zs_h = nc.dram_tensor("zs", [P, NSC, 2, SC, NC], mybir.dt.float16, kind="ExternalInput")
    bias_h = nc.dram_tensor("bias", [P, NC], mybir.dt.float32, kind="ExternalInput")
    out_h = nc.dram_tensor("out", [M, NC], mybir.dt.float16, kind="ExternalOutput")

    with tile.TileContext(nc) as tc:
        with (
            tc.tile_pool(name="singles", bufs=1) as singles,
            tc.tile_pool(name="qbp", bufs=4) as qbp,
            tc.tile_pool(name="zsp", bufs=NSC) as zsp,
            tc.tile_pool(name="wpool", bufs=NSC) as wpool,
            tc.tile_pool(name="xp", bufs=5) as xp,
            tc.tile_pool(name="psum", bufs=8, space="PSUM") as psum,
            tc.tile_pool(name="op", bufs=8) as op,
        ):
            # ---- sync-ring FIFO: xt0, then all dequant inputs, then xt1+ ----
            xt_tiles = []

            def load_xt(mt):
                xt = xp.tile([P, G, MT], mybir.dt.float16, tag="xt",
                             name=f"xt{mt}")
                nc.sync.dma_start_transpose(
                    xt[:], x_h.ap()[mt * MT : (mt + 1) * MT, :]
                )
                xt_tiles.append(xt)

            load_xt(0)

            qb_tiles, zs_tiles = [], []
            for sci in range(NSC):
                qb_t = qbp.tile([P, SC * NC], mybir.dt.uint8, tag="qb")
                nc.sync.dma_start(
                    qb_t[:], qb_h.ap()[:, sci * SC * NC : (sci + 1) * SC * NC]
                )
                qb_tiles.append(qb_t)
                zs_t = zsp.tile([P, 2, SC, NC], mybir.dt.float16, tag="zs")
                nc.sync.dma_start(zs_t[:], zs_h.ap()[:, sci, :, :, :])
                zs_tiles.append(zs_t)
            bias_t = singles.tile([P, NC], mybir.dt.float32)
            nc.sync.dma_start(bias_t[:], bias_h.ap())

            for mt in range(1, NMT):
                load_xt(mt)

            # ---- dequantize W: w = cast_f16(q) * s - t, per super-chunk.
            # All casts are emitted first: they only need qb tiles, so they
            # stream behind the qb loads while zs loads are still landing. ----
            W_tiles = []
            for sci in range(NSC):
                w_t = wpool.tile([P, SC, NC], mybir.dt.float16, tag="W",
                                 name=f"W{sci}")
                nc.vector.tensor_copy(
                    out=w_t[:].rearrange("p a b -> p (a b)"), in_=qb_tiles[sci][:]
                )
                W_tiles.append(w_t)
            for sci in range(NSC):
                w_t = W_tiles[sci]
                nc.vector.tensor_tensor(
                    w_t[:], w_t[:], zs_tiles[sci][:, 0, :, :], mybir.AluOpType.mult
                )
                nc.vector.tensor_tensor(
                    w_t[:], w_t[:], zs_tiles[sci][:, 1, :, :], mybir.AluOpType.subtract
                )

            # fp8 copies of the last G8 chunks of W (same values rounded to
            # e4m3; scales/zeros already folded in). Pairs are SC-aligned for
            # even G16, so each pair lives in one W tile.
            w8_t = None
            if R_FP8:
                w8_t = singles.tile([P, R_FP8, 2, NC], mybir.dt.float8e4)
                for j in range(R_FP8):
                    sci, off = divmod(G16 + 2 * j, SC)
                    nc.vector.tensor_copy(
                        out=w8_t[:, j, :, :],
                        in_=W_tiles[sci][:, off : off + 2, :],
                    )

            def x8_cast(xt, mt):
                """e4m3 copy of the last G8 chunks of this x tile."""
                if not R_FP8:
                    return None
                x8 = op.tile([P, R_FP8, 2, MT], mybir.dt.float8e4, tag="x8",
                             name=f"x8_{mt}")
                nc.vector.tensor_copy(
                    out=x8[:].rearrange("p a b c -> p (a b c)"),
                    in_=xt[:, G16:G, :].rearrange("p a b -> p (a b)"),
                )
                return x8

            # ---- matmuls ----
            def epilogue(ps, m0, store_eng):
                ob = op.tile([P, NC], mybir.dt.float16)
                nc.vector.tensor_tensor(ob[:], ps[:], bias_t[:], mybir.AluOpType.add)
                store_eng.dma_start(out_h.ap()[m0 : m0 + P, :], ob[:])

            def fp8_tail(ps, x8, sub):
                for j in range(R_FP8):
                    nc.tensor.matmul(
                        ps[:],
                        x8[:, j, :, sub * P : (sub + 1) * P],
                        w8_t[:, j, :, :],
                        start=False,
                        stop=(j == R_FP8 - 1),
                        perf_mode=mybir.MatmulPerfMode.DoubleRow,
                    )

            # first two m-tiles: super-chunk-interleaved accumulation so the
            # PE tracks dequant progress instead of stalling on W chunk 31
            for mt in range(2):
                xt = xt_tiles[mt]
                x8 = x8_cast(xt, mt)
                pss = [psum.tile([P, NC], mybir.dt.float32, tag="ps",
                                 name=f"ps_a{mt}_{i}")
                       for i in range(MSUB)]
                for sci in range(NSC):
                    for sub in range(MSUB):
                        for gi in range(SC):
                            g = sci * SC + gi
                            if g >= G16:
                                continue
                            nc.tensor.matmul(
                                pss[sub][:],
                                xt[:, g, sub * P : (sub + 1) * P],
                                W_tiles[sci][:, gi, :],
                                start=(sci == 0 and gi == 0),
                                stop=(G8 == 0 and sci == NSC - 1 and gi == SC - 1),
                            )
                for sub in range(MSUB):
                    if R_FP8:
                        fp8_tail(pss[sub], x8, sub)
                    # stores ride SWDGE: they serialize one-way against the
                    # transposes (store waits, transpose train unaffected),
                    # unlike HWDGE stores whose guard-waits feed back into
                    # the transpose cadence.
                    epilogue(pss[sub], mt * MT + sub * P, nc.gpsimd)

            # remaining m-tiles: plain k-ascending accumulation
            for mt in range(2, NMT):
                xt = xt_tiles[mt]
                x8 = x8_cast(xt, mt)
                for sub in range(MSUB):
                    ps = psum.tile([P, NC], mybir.dt.float32, tag="ps")
                    for g in range(G16):
                        nc.tensor.matmul(
                            ps[:],
                            xt[:, g, sub * P : (sub + 1) * P],
                            W_tiles[g // SC][:, g % SC, :],
                            start=(g == 0),
                            stop=(G8 == 0 and g == G16 - 1),
                        )
                    if R_FP8:
                        fp8_tail(ps, x8, sub)
                    # last tile's store rides the (by now idle) sync ring so
                    # its completion isn't stuck behind queued SWDGE stores.
                    store_eng = nc.sync if mt == NMT - 1 else nc.gpsimd
                    epilogue(ps, mt * MT + sub * P, store_eng)

    _split_multiwaits(nc)
    _built = nc
    return nc


def _host_prep(x, qweight, qzeros, scales, bias):
    """Host-side slicing + layout prep (nibble unpack to u8, zeros fold)."""
    x2d = np.ascontiguousarray(np.asarray(x).reshape(M, K))
    qweight = np.asarray(qweight)
    qzeros = np.asarray(qzeros)
    scales = np.asarray(scales)
    bias = np.asarray(bias)

    sh8 = (4 * np.arange(8, dtype=np.int32))[None, :, None]
    # zeros: [G, N] fp32; GPTQ stores z-1
    z = (((qzeros.astype(np.int64)[:, :, None] >> (4 * np.arange(8, dtype=np.int64))[None, None, :]) & 0xF)
         .reshape(G, N) + 1).astype(np.float32)
    s32 = scales.astype(np.float32)
    t_full = (z * s32).astype(np.float16)          # [G, N]
    s_full = scales.astype(np.float16)             # [G, N]

    # nibble-extract all of qweight once: [K, N] u8, k = 8*r + j
    q8_full = ((qweight[:, None, :] >> sh8) & 0xF).astype(np.uint8).reshape(K, N)

    in_maps = []
    for c in range(NCORES):
        n0 = c * NC
        # qb[p, g*NC + n] = q8[g*128 + p, n0 + n]
        qb = np.ascontiguousarray(
            q8_full[:, n0 : n0 + NC].reshape(G, P, NC).transpose(1, 0, 2)
            .reshape(P, G * NC)
        )
        zs = np.empty((NSC, 2, SC, NC), dtype=np.float16)
        zs[:, 0] = s_full[:, n0 : n0 + NC].reshape(NSC, SC, NC)
        zs[:, 1] = t_full[:, n0 : n0 + NC].reshape(NSC, SC, NC)
        zs_rep = np.ascontiguousarray(
            np.broadcast_to(zs[None], (P, NSC, 2, SC, NC))
        )
        bias_rep = np.ascontiguousarray(
            np.broadcast_to(
                bias[n0 : n0 + NC].astype(np.float32)[None], (P, NC)
            )
        )
        in_maps.append({"x": x2d, "qb": qb, "zs": zs_rep, "bias": bias_rep})
    return in_maps


def run(inputs, trace=False, **spmd_kwargs):
    """Run on 8 cores; returns (full_output [4,2048,4096] fp16, BassKernelResults)."""
    nc = _build_bass()
    in_maps = _host_prep(
        inputs["x"], inputs["qweight"], inputs["qzeros"], inputs["scales"],
        inputs["bias"],
    )
    res = run_bass_kernel_spmd(
        nc, in_maps, core_ids=list(range(NCORES)), trace=trace, **spmd_kwargs
    )
    out = np.concatenate([r["out"] for r in res.results], axis=1)
    out = out.reshape(B, S, N).astype(np.float16)
    return out, res


def kernel(x, qweight, qzeros, scales, g_idx, bias):
    out, _ = run(
        {"x": x, "qweight": qweight, "qzeros": qzeros, "scales": scales, "bias": bias}
    )
    return out


# revision 40
# speedup vs baseline: 1.0376x; 1.0376x over previous
"""GPTQ/ExLlama 4-bit grouped-quantized linear on 8 Trainium2 NeuronCores.

out = x @ dequant(qweight, qzeros, scales) + bias
  x: [4, 2048, 4096] fp16, qweight: [512, 4096] int32 (8 nibbles/int32 along K),
  qzeros: [32, 512] int32 (8 nibbles/int32 along N), scales: [32, 4096] fp16,
  g_idx = arange(K)//128, bias: [4096] fp16.

Sharding: Megatron column-parallel. Each of the 8 cores gets the full x
(replicated) and a 512-wide column slice of qweight/zeros/scales/bias, computes
out[:, n_slice] = x @ W[:, n_slice] + bias[n_slice]; the host concatenates.

Host prep (layout only): nibbles are pre-extracted to uint8 (qb[p, g, n] =
nibble k=g*128+p of column n) so the device does no shift/mask work, and the
zeros are folded as t = (z+1)*s so dequant is the 3-op chain
  w = cast_f16(q) * s - t
with s/t host-replicated across partitions (plain fast linear loads).

Per-core kernel schedule. Steady state runs at the PE roofline (216 ns per
128x128x512 fp16 matmul, LDWEIGHTS hidden by the reorder window); startup is
dominated by the tile scheduler's DMA-transpose deadlock guard, which
bidirectionally serializes ANY concurrently-scheduled DMA against an
in-flight transpose (and a waiting DMA head-of-line blocks its whole engine
queue). Hence:
  - sync (SP) HWDGE ring, strict FIFO: xt0 transpose, then ALL dequant
    inputs (qb/zs interleaved per super-chunk, then bias), then xt1..31.
    Same-ring ordering costs only bandwidth, never a guard cascade.
  - stores ride SWDGE (gpsimd): they serialize one-way against transposes
    (the store waits; the transpose train is unaffected). Last store rides
    sync so its completion isn't queued behind SWDGE at the kernel tail.
  - PE: the first two m-tiles' matmuls are emitted super-chunk-interleaved
    (accumulate k-chunks 4sc..4sc+3 across the in-flight PSUM banks as W
    chunks dequantize) so the PE absorbs the dequant tail with real work;
    remaining m-tiles in plain k-ascending order.
  - DVE: u8->f16 casts first (qb-gated), then mul/sub chains (zs-gated),
    then per-tile bias adds during the PSUM->SBUF drain.
fp8 DoubleRow was tried for the last 6 k-chunks (rel err 1.65e-2, passes) but
measured at parity with fp16 (DoubleRow LDWEIGHTS can't pull ahead), so it
stays disabled (R_FP8 = 0).
"""

import os
import sys

for _p in ("/opt/trn_rl_repo", "/root/.axon_site/_ro/trn_rl_repo"):
    if os.path.isdir(_p) and _p not in sys.path:
        sys.path.insert(0, _p)

import numpy as np

import concourse.bass as bass
import concourse.mybir as mybir
import concourse.tile as tile
from concourse.bass_utils import run_bass_kernel_spmd

P = 128                    # partitions
B, S, K, N = 4, 2048, 4096, 4096
M = B * S                  # 8192 rows
GS = 128                   # quant group size (== one k-chunk)
G = K // GS                # 32 groups == k-chunks
NCORES = 8
NC = N // NCORES           # 512 output cols per core
SC = 4                     # groups per dequant super-chunk
NSC = G // SC              # 8 super-chunks
MT = 256                   # x rows per transposed DMA load
NMT = M // MT              # 32 loads
MSUB = MT // P             # 2 psum tiles per load
NWARM = 24                 # PE warm-up matmuls

# fp8 hybrid: the last 2*R_FP8 k-chunks run as e4m3 DoubleRow matmul pairs.
# Measured end-to-end rel-err vs the reference (seed-0): R=0: 5.7e-4,
# R=3: 1.65e-2 (passes the 2e-2 gate) — but on HW a DoubleRow pair costs
# ~375-430 ns vs 432 ns for the two fp16 matmuls it replaces (the
# interleaved LDWEIGHTS can't pull ahead into the background weight buffer,
# so its 213 ns serializes). No net win, so keep fp16 everywhere.
R_FP8 = 0
G8 = 2 * R_FP8             # fp8 chunks
G16 = G - G8               # fp16 chunks

_built = None


def _split_multiwaits(nc):
    """This container's walrus rejects any instruction carrying more than one
    semaphore wait ("Too many sync wait commands"). Hoist all but one wait of
    each multi-wait instruction into standalone EventSemaphore (wait-only)
    instructions on the same engine, inserted immediately before it — the
    engine queue is FIFO, so semantics are identical."""
    n = 0
    for fn in nc.m.functions:
        for blk in fn.blocks:
            out = []
            for inst in blk.instructions:
                si = getattr(inst, "sync_info", None)
                waits = list(si.on_wait) if si is not None and si.on_wait else []
                if len(waits) > 1:
                    for k, w in enumerate(waits[:-1]):
                        es = mybir.InstEventSemaphore(
                            name=f"{inst.name}.hoistw{k}", ins=[], outs=[],
                            sync_info=mybir.SyncInfo(on_wait=[w], on_update=[]),
                        )
                        es.engine = inst.engine
                        out.append(es)
                        n += 1
                    si.on_wait = [waits[-1]]
                out.append(inst)
            blk.instructions = out
    return n


def _build_bass():
    """Build the (identical-per-core) Bass program once."""
    global _built
    if _built is not None:
        return _built

    nc = bass.Bass()
    x_h = nc.dram_tensor("x", [M, K], mybir.dt.float16, kind="ExternalInput")
    qb_h = nc.dram_tensor("qb", [P, G * NC], mybir.dt.uint8, kind="ExternalInput")
    # zs/bias come host-replicated across partitions. All dequant inputs ride
    # the SAME ring as the x transposes, in FIFO order right after xt0: any
    # DMA scheduled on another ring while a transpose is in flight gets
    # serialized against that whole transpose by the tile scheduler's
    # deadlock guard (and a waiting DMA head-of-line blocks its engine's
    # entire queue), which is far worse than plain bandwidth sharing.
    zs_h = nc.dram_tensor("zs", [P, NSC, 2, SC, NC], mybir.dt.float16, kind="ExternalInput")
    bias_h = nc.dram_tensor("bias", [P, NC], mybir.dt.float32, kind="ExternalInput")
    out_h = nc.dram_tensor("out", [M, NC], mybir.dt.float16, kind="ExternalOutput")

    with tile.TileContext(nc) as tc:
        with (
            tc.tile_pool(name="singles", bufs=1) as singles,
            tc.tile_pool(name="qbp", bufs=4) as qbp,
            tc.tile_pool(name="zsp", bufs=NSC) as zsp,
            tc.tile_pool(name="wpool", bufs=NSC) as wpool,
            tc.tile_pool(name="xp", bufs=5) as xp,
            tc.tile_pool(name="psum", bufs=8, space="PSUM") as psum,
            tc.tile_pool(name="op", bufs=8) as op,
        ):
            # ---- sync-ring FIFO: qb0+zs0 (so the first dequant chain starts
            # before xt0's 10 us transpose completes), xt0, the remaining
            # dequant inputs, then xt1..31. ALL transposes stay on one ring:
            # concurrent transposes on both HWDGE rings corrupt data (the
            # XBAR is one shared resource — measured rel err 0.28). ----
            xt_tiles = []

            def load_xt(mt):
                xt = xp.tile([P, G, MT], mybir.dt.float16, tag="xt",
                             name=f"xt{mt}")
                nc.sync.dma_start_transpose(
                    xt[:], x_h.ap()[mt * MT : (mt + 1) * MT, :]
                )
                xt_tiles.append(xt)

            qb_tiles, zs_tiles = [], []

            def load_sc(sci):
                qb_t = qbp.tile([P, SC * NC], mybir.dt.uint8, tag="qb")
                nc.sync.dma_start(
                    qb_t[:], qb_h.ap()[:, sci * SC * NC : (sci + 1) * SC * NC]
                )
                qb_tiles.append(qb_t)
                zs_t = zsp.tile([P, 2, SC, NC], mybir.dt.float16, tag="zs")
                nc.sync.dma_start(zs_t[:], zs_h.ap()[:, sci, :, :, :])
                zs_tiles.append(zs_t)

            load_sc(0)
            load_xt(0)
            for sci in range(1, NSC):
                load_sc(sci)
            bias_t = singles.tile([P, NC], mybir.dt.float32)
            nc.sync.dma_start(bias_t[:], bias_h.ap())

            for mt in range(1, NMT):
                load_xt(mt)

            # ---- dequantize W: w = cast_f16(q) * s - t, per super-chunk.
            # All casts are emitted first: they only need qb tiles, so they
            # stream behind the qb loads while zs loads are still landing. ----
            W_tiles = []
            for sci in range(NSC):
                w_t = wpool.tile([P, SC, NC], mybir.dt.float16, tag="W",
                                 name=f"W{sci}")
                nc.vector.tensor_copy(
                    out=w_t[:].rearrange("p a b -> p (a b)"), in_=qb_tiles[sci][:]
                )
                W_tiles.append(w_t)
            for sci in range(NSC):
                w_t = W_tiles[sci]
                nc.vector.tensor_tensor(
                    w_t[:], w_t[:], zs_tiles[sci][:, 0, :, :], mybir.AluOpType.mult
                )
                nc.vector.tensor_tensor(
                    w_t[:], w_t[:], zs_tiles[sci][:, 1, :, :], mybir.AluOpType.subtract
                )

            # fp8 copies of the last G8 chunks of W (same values rounded to
            # e4m3; scales/zeros already folded in). Pairs are SC-aligned for
            # even G16, so each pair lives in one W tile.
            w8_t = None
            if R_FP8:
                w8_t = singles.tile([P, R_FP8, 2, NC], mybir.dt.float8e4)
                for j in range(R_FP8):
                    sci, off = divmod(G16 + 2 * j, SC)
                    nc.vector.tensor_copy(
                        out=w8_t[:, j, :, :],
                        in_=W_tiles[sci][:, off : off + 2, :],
                    )

            def x8_cast(xt, mt):
                """e4m3 copy of the last G8 chunks of this x tile."""
                if not R_FP8:
                    return None
                x8 = op.tile([P, R_FP8, 2, MT], mybir.dt.float8e4, tag="x8",
                             name=f"x8_{mt}")
                nc.vector.tensor_copy(
                    out=x8[:].rearrange("p a b c -> p (a b c)"),
                    in_=xt[:, G16:G, :].rearrange("p a b -> p (a b)"),
                )
                return x8

            # ---- matmuls ----
            def epilogue(ps, m0, store_eng):
                ob = op.tile([P, NC], mybir.dt.float16)
                nc.vector.tensor_tensor(ob[:], ps[:], bias_t[:], mybir.AluOpType.add)
                store_eng.dma_start(out_h.ap()[m0 : m0 + P, :], ob[:])

            def fp8_tail(ps, x8, sub):
                for j in range(R_FP8):
                    nc.tensor.matmul(
                        ps[:],
                        x8[:, j, :, sub * P : (sub + 1) * P],
                        w8_t[:, j, :, :],
                        start=False,
                        stop=(j == R_FP8 - 1),
                        perf_mode=mybir.MatmulPerfMode.DoubleRow,
                    )

            # first two m-tiles: super-chunk-interleaved accumulation so the
            # PE tracks dequant progress instead of stalling on W chunk 31
            for mt in range(2):
                xt = xt_tiles[mt]
                x8 = x8_cast(xt, mt)
                pss = [psum.tile([P, NC], mybir.dt.float32, tag="ps",
                                 name=f"ps_a{mt}_{i}")
                       for i in range(MSUB)]
                for sci in range(NSC):
                    for sub in range(MSUB):
                        for gi in range(SC):
                            g = sci * SC + gi
                            if g >= G16:
                                continue
                            nc.tensor.matmul(
                                pss[sub][:],
                                xt[:, g, sub * P : (sub + 1) * P],
                                W_tiles[sci][:, gi, :],
                                start=(sci == 0 and gi == 0),
                                stop=(G8 == 0 and sci == NSC - 1 and gi == SC - 1),
                            )
                for sub in range(MSUB):
                    if R_FP8:
                        fp8_tail(pss[sub], x8, sub)
                    # stores ride SWDGE: they serialize one-way against the
                    # transposes (store waits, transpose train unaffected),
                    # unlike HWDGE stores whose guard-waits feed back into
                    # the transpose cadence.
                    epilogue(pss[sub], mt * MT + sub * P, nc.gpsimd)

            # remaining m-tiles: plain k-ascending accumulation
            for mt in range(2, NMT):
                xt = xt_tiles[mt]
                x8 = x8_cast(xt, mt)
                for sub in range(MSUB):
                    ps = psum.tile([P, NC], mybir.dt.float32, tag="ps")
                    for g in range(G16):
                        nc.tensor.matmul(
                            ps[:],
                            xt[:, g, sub * P : (sub + 1) * P],
                            W_tiles[g // SC][:, g % SC, :],
                            start=(g == 0),
                            stop=(G8 == 0 and g == G16 - 1),
                        )
                    if R_FP8:
                        fp8_tail(ps, x8, sub)
                    # last tile's store rides the (by now idle) sync ring so
                    # its completion isn't stuck behind queued SWDGE stores.
                    store_eng = nc.sync if mt == NMT - 1 else nc.gpsimd
                    epilogue(ps, mt * MT + sub * P, store_eng)

    _split_multiwaits(nc)
    _built = nc
    return nc


def _host_prep(x, qweight, qzeros, scales, bias):
    """Host-side slicing + layout prep (nibble unpack to u8, zeros fold)."""
    x2d = np.ascontiguousarray(np.asarray(x).reshape(M, K))
    qweight = np.asarray(qweight)
    qzeros = np.asarray(qzeros)
    scales = np.asarray(scales)
    bias = np.asarray(bias)

    sh8 = (4 * np.arange(8, dtype=np.int32))[None, :, None]
    # zeros: [G, N] fp32; GPTQ stores z-1
    z = (((qzeros.astype(np.int64)[:, :, None] >> (4 * np.arange(8, dtype=np.int64))[None, None, :]) & 0xF)
         .reshape(G, N) + 1).astype(np.float32)
    s32 = scales.astype(np.float32)
    t_full = (z * s32).astype(np.float16)          # [G, N]
    s_full = scales.astype(np.float16)             # [G, N]

    # nibble-extract all of qweight once: [K, N] u8, k = 8*r + j
    q8_full = ((qweight[:, None, :] >> sh8) & 0xF).astype(np.uint8).reshape(K, N)

    in_maps = []
    for c in range(NCORES):
        n0 = c * NC
        # qb[p, g*NC + n] = q8[g*128 + p, n0 + n]
        qb = np.ascontiguousarray(
            q8_full[:, n0 : n0 + NC].reshape(G, P, NC).transpose(1, 0, 2)
            .reshape(P, G * NC)
        )
        zs = np.empty((NSC, 2, SC, NC), dtype=np.float16)
        zs[:, 0] = s_full[:, n0 : n0 + NC].reshape(NSC, SC, NC)
        zs[:, 1] = t_full[:, n0 : n0 + NC].reshape(NSC, SC, NC)
        zs_rep = np.ascontiguousarray(
            np.broadcast_to(zs[None], (P, NSC, 2, SC, NC))
        )
        bias_rep = np.ascontiguousarray(
            np.broadcast_to(
                bias[n0 : n0 + NC].astype(np.float32)[None], (P, NC)
            )
        )
        in_maps.append({"x": x2d, "qb": qb, "zs": zs_rep, "bias": bias_rep})
    return in_maps


def run(inputs, trace=False, **spmd_kwargs):
    """Run on 8 cores; returns (full_output [4,2048,4096] fp16, BassKernelResults)."""
    nc = _build_bass()
    in_maps = _host_prep(
        inputs["x"], inputs["qweight"], inputs["qzeros"], inputs["scales"],
        inputs["bias"],
    )
    res = run_bass_kernel_spmd(
        nc, in_maps, core_ids=list(range(NCORES)), trace=trace, **spmd_kwargs
    )
    out = np.concatenate([r["out"] for r in res.results], axis=1)
    out = out.reshape(B, S, N).astype(np.float16)
    return out, res


def kernel(x, qweight, qzeros, scales, g_idx, bias):
    out, _ = run(
        {"x": x, "qweight": qweight, "qzeros": qzeros, "scales": scales, "bias": bias}
    )
    return out


# revision 41
# speedup vs baseline: 1.0490x; 1.0109x over previous
"""GPTQ/ExLlama 4-bit grouped-quantized linear on 8 Trainium2 NeuronCores.

out = x @ dequant(qweight, qzeros, scales) + bias
  x: [4, 2048, 4096] fp16, qweight: [512, 4096] int32 (8 nibbles/int32 along K),
  qzeros: [32, 512] int32 (8 nibbles/int32 along N), scales: [32, 4096] fp16,
  g_idx = arange(K)//128, bias: [4096] fp16.

Sharding: Megatron column-parallel. Each of the 8 cores gets the full x
(replicated) and a 512-wide column slice of qweight/zeros/scales/bias, computes
out[:, n_slice] = x @ W[:, n_slice] + bias[n_slice]; the host concatenates.

Host prep (layout only): nibbles are pre-extracted to uint8 (qb[p, g, n] =
nibble k=g*128+p of column n) so the device does no shift/mask work, and the
zeros are folded as t = (z+1)*s so dequant is the 3-op chain
  w = cast_f16(q) * s - t
with s/t host-replicated across partitions (plain fast linear loads).

Per-core kernel schedule. Steady state runs at the PE roofline (216 ns per
128x128x512 fp16 matmul, LDWEIGHTS hidden by the reorder window); startup is
dominated by the tile scheduler's DMA-transpose deadlock guard, which
bidirectionally serializes ANY concurrently-scheduled DMA against an
in-flight transpose (and a waiting DMA head-of-line blocks its whole engine
queue). Hence:
  - sync (SP) HWDGE ring, strict FIFO: xt0 transpose, then ALL dequant
    inputs (qb/zs interleaved per super-chunk, then bias), then xt1..31.
    Same-ring ordering costs only bandwidth, never a guard cascade.
  - stores ride SWDGE (gpsimd): they serialize one-way against transposes
    (the store waits; the transpose train is unaffected). Last store rides
    sync so its completion isn't queued behind SWDGE at the kernel tail.
  - PE: the first two m-tiles' matmuls are emitted super-chunk-interleaved
    (accumulate k-chunks 4sc..4sc+3 across the in-flight PSUM banks as W
    chunks dequantize) so the PE absorbs the dequant tail with real work;
    remaining m-tiles in plain k-ascending order.
  - DVE: u8->f16 casts first (qb-gated), then mul/sub chains (zs-gated),
    then per-tile bias adds during the PSUM->SBUF drain.
fp8 DoubleRow was tried for the last 6 k-chunks (rel err 1.65e-2, passes) but
measured at parity with fp16 (DoubleRow LDWEIGHTS can't pull ahead), so it
stays disabled (R_FP8 = 0).
"""

import os
import sys

for _p in ("/opt/trn_rl_repo", "/root/.axon_site/_ro/trn_rl_repo"):
    if os.path.isdir(_p) and _p not in sys.path:
        sys.path.insert(0, _p)

import numpy as np

import concourse.bass as bass
import concourse.mybir as mybir
import concourse.tile as tile
from concourse.bass_utils import run_bass_kernel_spmd

P = 128                    # partitions
B, S, K, N = 4, 2048, 4096, 4096
M = B * S                  # 8192 rows
GS = 128                   # quant group size (== one k-chunk)
G = K // GS                # 32 groups == k-chunks
NCORES = 8
NC = N // NCORES           # 512 output cols per core
SC = 4                     # groups per dequant super-chunk
NSC = G // SC              # 8 super-chunks
MT = 256                   # x rows per transposed DMA load
NMT = M // MT              # 32 loads
MSUB = MT // P             # 2 psum tiles per load
NWARM = 24                 # PE warm-up matmuls

# fp8 hybrid: the last 2*R_FP8 k-chunks run as e4m3 DoubleRow matmul pairs.
# Measured end-to-end rel-err vs the reference (seed-0): R=0: 5.7e-4,
# R=3: 1.65e-2 (passes the 2e-2 gate) — but on HW a DoubleRow pair costs
# ~375-430 ns vs 432 ns for the two fp16 matmuls it replaces (the
# interleaved LDWEIGHTS can't pull ahead into the background weight buffer,
# so its 213 ns serializes). No net win, so keep fp16 everywhere.
R_FP8 = 0
G8 = 2 * R_FP8             # fp8 chunks
G16 = G - G8               # fp16 chunks

_built = None


def _split_multiwaits(nc):
    """This container's walrus rejects any instruction carrying more than one
    semaphore wait ("Too many sync wait commands"). Hoist all but one wait of
    each multi-wait instruction into standalone EventSemaphore (wait-only)
    instructions on the same engine, inserted immediately before it — the
    engine queue is FIFO, so semantics are identical."""
    n = 0
    for fn in nc.m.functions:
        for blk in fn.blocks:
            out = []
            for inst in blk.instructions:
                si = getattr(inst, "sync_info", None)
                waits = list(si.on_wait) if si is not None and si.on_wait else []
                if len(waits) > 1:
                    for k, w in enumerate(waits[:-1]):
                        es = mybir.InstEventSemaphore(
                            name=f"{inst.name}.hoistw{k}", ins=[], outs=[],
                            sync_info=mybir.SyncInfo(on_wait=[w], on_update=[]),
                        )
                        es.engine = inst.engine
                        out.append(es)
                        n += 1
                    si.on_wait = [waits[-1]]
                out.append(inst)
            blk.instructions = out
    return n


def _build_bass():
    """Build the (identical-per-core) Bass program once."""
    global _built
    if _built is not None:
        return _built

    nc = bass.Bass()
    x_h = nc.dram_tensor("x", [M, K], mybir.dt.float16, kind="ExternalInput")
    qb_h = nc.dram_tensor("qb", [P, G * NC], mybir.dt.uint8, kind="ExternalInput")
    # zs/bias come host-replicated across partitions. All dequant inputs ride
    # the SAME ring as the x transposes, in FIFO order right after xt0: any
    # DMA scheduled on another ring while a transpose is in flight gets
    # serialized against that whole transpose by the tile scheduler's
    # deadlock guard (and a waiting DMA head-of-line blocks its engine's
    # entire queue), which is far worse than plain bandwidth sharing.
    zs_h = nc.dram_tensor("zs", [P, NSC, 2, SC, NC], mybir.dt.float16, kind="ExternalInput")
    bias_h = nc.dram_tensor("bias", [P, NC], mybir.dt.float32, kind="ExternalInput")
    out_h = nc.dram_tensor("out", [M, NC], mybir.dt.float16, kind="ExternalOutput")

    with tile.TileContext(nc) as tc:
        with (
            tc.tile_pool(name="singles", bufs=1) as singles,
            tc.tile_pool(name="qbp", bufs=4) as qbp,
            tc.tile_pool(name="zsp", bufs=NSC) as zsp,
            tc.tile_pool(name="wpool", bufs=NSC) as wpool,
            tc.tile_pool(name="xp", bufs=5) as xp,
            tc.tile_pool(name="psum", bufs=8, space="PSUM") as psum,
            tc.tile_pool(name="op", bufs=8) as op,
        ):
            # ---- sync-ring FIFO: qb0+zs0 (so the first dequant chain starts
            # before xt0's 10 us transpose completes), xt0, the remaining
            # dequant inputs, then xt1..31. ALL transposes stay on one ring:
            # concurrent transposes on both HWDGE rings corrupt data (the
            # XBAR is one shared resource — measured rel err 0.28). ----
            xt_tiles = []

            def load_xt(mt):
                xt = xp.tile([P, G, MT], mybir.dt.float16, tag="xt",
                             name=f"xt{mt}")
                nc.sync.dma_start_transpose(
                    xt[:], x_h.ap()[mt * MT : (mt + 1) * MT, :]
                )
                xt_tiles.append(xt)

            qb_tiles, zs_tiles = [], []

            def load_sc(sci):
                qb_t = qbp.tile([P, SC * NC], mybir.dt.uint8, tag="qb")
                nc.sync.dma_start(
                    qb_t[:], qb_h.ap()[:, sci * SC * NC : (sci + 1) * SC * NC]
                )
                qb_tiles.append(qb_t)
                zs_t = zsp.tile([P, 2, SC, NC], mybir.dt.float16, tag="zs")
                nc.sync.dma_start(zs_t[:], zs_h.ap()[:, sci, :, :, :])
                zs_tiles.append(zs_t)

            # Two super-chunks land before xt0 (PE has W0/W1 runway the
            # moment xt0 arrives); xt1 slots in after SC5 so phase B isn't
            # starved while SC6/SC7 still cover phase A's tail.
            load_sc(0)
            load_sc(1)
            load_xt(0)
            for sci in range(2, 6):
                load_sc(sci)
            load_xt(1)
            for sci in range(6, NSC):
                load_sc(sci)
            bias_t = singles.tile([P, NC], mybir.dt.float32)
            nc.sync.dma_start(bias_t[:], bias_h.ap())

            for mt in range(2, NMT):
                load_xt(mt)

            # ---- dequantize W: w = cast_f16(q) * s - t, per super-chunk.
            # All casts are emitted first: they only need qb tiles, so they
            # stream behind the qb loads while zs loads are still landing. ----
            W_tiles = []
            for sci in range(NSC):
                w_t = wpool.tile([P, SC, NC], mybir.dt.float16, tag="W",
                                 name=f"W{sci}")
                nc.vector.tensor_copy(
                    out=w_t[:].rearrange("p a b -> p (a b)"), in_=qb_tiles[sci][:]
                )
                W_tiles.append(w_t)
            for sci in range(NSC):
                w_t = W_tiles[sci]
                nc.vector.tensor_tensor(
                    w_t[:], w_t[:], zs_tiles[sci][:, 0, :, :], mybir.AluOpType.mult
                )
                nc.vector.tensor_tensor(
                    w_t[:], w_t[:], zs_tiles[sci][:, 1, :, :], mybir.AluOpType.subtract
                )

            # fp8 copies of the last G8 chunks of W (same values rounded to
            # e4m3; scales/zeros already folded in). Pairs are SC-aligned for
            # even G16, so each pair lives in one W tile.
            w8_t = None
            if R_FP8:
                w8_t = singles.tile([P, R_FP8, 2, NC], mybir.dt.float8e4)
                for j in range(R_FP8):
                    sci, off = divmod(G16 + 2 * j, SC)
                    nc.vector.tensor_copy(
                        out=w8_t[:, j, :, :],
                        in_=W_tiles[sci][:, off : off + 2, :],
                    )

            def x8_cast(xt, mt):
                """e4m3 copy of the last G8 chunks of this x tile."""
                if not R_FP8:
                    return None
                x8 = op.tile([P, R_FP8, 2, MT], mybir.dt.float8e4, tag="x8",
                             name=f"x8_{mt}")
                nc.vector.tensor_copy(
                    out=x8[:].rearrange("p a b c -> p (a b c)"),
                    in_=xt[:, G16:G, :].rearrange("p a b -> p (a b)"),
                )
                return x8

            # ---- matmuls ----
            def epilogue(ps, m0, store_eng):
                ob = op.tile([P, NC], mybir.dt.float16)
                nc.vector.tensor_tensor(ob[:], ps[:], bias_t[:], mybir.AluOpType.add)
                store_eng.dma_start(out_h.ap()[m0 : m0 + P, :], ob[:])

            def fp8_tail(ps, x8, sub):
                for j in range(R_FP8):
                    nc.tensor.matmul(
                        ps[:],
                        x8[:, j, :, sub * P : (sub + 1) * P],
                        w8_t[:, j, :, :],
                        start=False,
                        stop=(j == R_FP8 - 1),
                        perf_mode=mybir.MatmulPerfMode.DoubleRow,
                    )

            # first two m-tiles: super-chunk-interleaved accumulation so the
            # PE tracks dequant progress instead of stalling on W chunk 31
            for mt in range(2):
                xt = xt_tiles[mt]
                x8 = x8_cast(xt, mt)
                pss = [psum.tile([P, NC], mybir.dt.float32, tag="ps",
                                 name=f"ps_a{mt}_{i}")
                       for i in range(MSUB)]
                for sci in range(NSC):
                    for sub in range(MSUB):
                        for gi in range(SC):
                            g = sci * SC + gi
                            if g >= G16:
                                continue
                            nc.tensor.matmul(
                                pss[sub][:],
                                xt[:, g, sub * P : (sub + 1) * P],
                                W_tiles[sci][:, gi, :],
                                start=(sci == 0 and gi == 0),
                                stop=(G8 == 0 and sci == NSC - 1 and gi == SC - 1),
                            )
                for sub in range(MSUB):
                    if R_FP8:
                        fp8_tail(pss[sub], x8, sub)
                    # stores ride SWDGE: they serialize one-way against the
                    # transposes (store waits, transpose train unaffected),
                    # unlike HWDGE stores whose guard-waits feed back into
                    # the transpose cadence.
                    epilogue(pss[sub], mt * MT + sub * P, nc.gpsimd)

            # remaining m-tiles: plain k-ascending accumulation
            for mt in range(2, NMT):
                xt = xt_tiles[mt]
                x8 = x8_cast(xt, mt)
                for sub in range(MSUB):
                    ps = psum.tile([P, NC], mybir.dt.float32, tag="ps")
                    for g in range(G16):
                        nc.tensor.matmul(
                            ps[:],
                            xt[:, g, sub * P : (sub + 1) * P],
                            W_tiles[g // SC][:, g % SC, :],
                            start=(g == 0),
                            stop=(G8 == 0 and g == G16 - 1),
                        )
                    if R_FP8:
                        fp8_tail(ps, x8, sub)
                    # last tile's store rides the (by now idle) sync ring so
                    # its completion isn't stuck behind queued SWDGE stores.
                    store_eng = nc.sync if mt == NMT - 1 else nc.gpsimd
                    epilogue(ps, mt * MT + sub * P, store_eng)

    _split_multiwaits(nc)
    _built = nc
    return nc


def _host_prep(x, qweight, qzeros, scales, bias):
    """Host-side slicing + layout prep (nibble unpack to u8, zeros fold)."""
    x2d = np.ascontiguousarray(np.asarray(x).reshape(M, K))
    qweight = np.asarray(qweight)
    qzeros = np.asarray(qzeros)
    scales = np.asarray(scales)
    bias = np.asarray(bias)

    sh8 = (4 * np.arange(8, dtype=np.int32))[None, :, None]
    # zeros: [G, N] fp32; GPTQ stores z-1
    z = (((qzeros.astype(np.int64)[:, :, None] >> (4 * np.arange(8, dtype=np.int64))[None, None, :]) & 0xF)
         .reshape(G, N) + 1).astype(np.float32)
    s32 = scales.astype(np.float32)
    t_full = (z * s32).astype(np.float16)          # [G, N]
    s_full = scales.astype(np.float16)             # [G, N]

    # nibble-extract all of qweight once: [K, N] u8, k = 8*r + j
    q8_full = ((qweight[:, None, :] >> sh8) & 0xF).astype(np.uint8).reshape(K, N)

    in_maps = []
    for c in range(NCORES):
        n0 = c * NC
        # qb[p, g*NC + n] = q8[g*128 + p, n0 + n]
        qb = np.ascontiguousarray(
            q8_full[:, n0 : n0 + NC].reshape(G, P, NC).transpose(1, 0, 2)
            .reshape(P, G * NC)
        )
        zs = np.empty((NSC, 2, SC, NC), dtype=np.float16)
        zs[:, 0] = s_full[:, n0 : n0 + NC].reshape(NSC, SC, NC)
        zs[:, 1] = t_full[:, n0 : n0 + NC].reshape(NSC, SC, NC)
        zs_rep = np.ascontiguousarray(
            np.broadcast_to(zs[None], (P, NSC, 2, SC, NC))
        )
        bias_rep = np.ascontiguousarray(
            np.broadcast_to(
                bias[n0 : n0 + NC].astype(np.float32)[None], (P, NC)
            )
        )
        in_maps.append({"x": x2d, "qb": qb, "zs": zs_rep, "bias": bias_rep})
    return in_maps


def run(inputs, trace=False, **spmd_kwargs):
    """Run on 8 cores; returns (full_output [4,2048,4096] fp16, BassKernelResults)."""
    nc = _build_bass()
    in_maps = _host_prep(
        inputs["x"], inputs["qweight"], inputs["qzeros"], inputs["scales"],
        inputs["bias"],
    )
    res = run_bass_kernel_spmd(
        nc, in_maps, core_ids=list(range(NCORES)), trace=trace, **spmd_kwargs
    )
    out = np.concatenate([r["out"] for r in res.results], axis=1)
    out = out.reshape(B, S, N).astype(np.float16)
    return out, res


def kernel(x, qweight, qzeros, scales, g_idx, bias):
    out, _ = run(
        {"x": x, "qweight": qweight, "qzeros": qzeros, "scales": scales, "bias": bias}
    )
    return out
